# revision 1
# baseline (speedup 1.0000x reference)
"""Trainium2 Bass kernel for the CPC loss problem (nn_CPC_85117661872355).

Strategy (data-parallel over batch B across 8 cores):
  - Each core handles 8 of the 64 batch elements: 1120 prediction rows.
  - pred = ctx @ Wk[s]^T + b on the PE as a 3-pass bf16 hi/lo split
    (cH*wH + cL*wH + cH*wL, fp32 PSUM accumulate) — full fp32-grade
    precision at bf16 matmul speed.  ctx^T and Wk^T are pre-transposed on
    the host so the contraction dim lands on partitions directly.
  - All 17 logits per row (1 positive + 16 negatives) are dot products
    pred_row . enc_flat[idx].  Target vectors are fetched with SWDGE
    dma_gather from an fp16 copy of the encoding table (halves gather
    bytes; logit noise ~7e-5 << typical top-2 gaps) and the dots are
    computed with the fused DVE scalar_tensor_tensor (mult+mult, accum)
    against the resident fp32 pred tile.  Gathering the positive through
    the same path keeps bitwise ties when a negative index collides with
    the positive, matching jnp.argmax's first-index tie-break.
  - Softmax-CE and the argmax==0 check run on ACT/DVE per 128-row
    supergroup; per-core (loss_sum, correct_sum) are reduced over
    partitions with a K=128 ones-matmul and DMA'd out as [1,2].
  - Host sums the 8 partial pairs and divides by n_preds.
"""

import functools

import ml_dtypes
import numpy as np

import concourse.bass as bass
import concourse.mybir as mybir
import concourse.tile as tile
from concourse import bacc
from concourse.bass_utils import run_bass_kernel_spmd

F32 = mybir.dt.float32
BF16 = mybir.dt.bfloat16
FP16 = mybir.dt.float16

B, G, D = 64, 7, 1280
S, NEG = 5, 16
NCORES = 8
BSH = B // NCORES  # 8
NS = [BSH * (6 - s) * G for s in range(S)]  # [336, 280, 224, 168, 112]
SOFF = [0]
for n in NS:
    SOFF.append(SOFF[-1] + n)
NR = SOFF[-1]  # 1120 rows per core
NSG = 9  # supergroups of 128 rows
SG_VALID = [128] * 8 + [96]
NDOT = 17  # 1 positive + 16 negatives
E_HALF = 640
T_S = [3, 3, 2, 2, 1]  # row-tiles per s
GCHUNKS = [(0, 4), (4, 4), (8, 4), (12, 4), (16, 1)]  # (goff, width)
IDX_PER_SG = NDOT * 128  # 2176
IDX_TOT = NSG * IDX_PER_SG  # 19584
N_PREDS = B * G * 20  # 8960

# Results of the last device run (for test harness introspection)
LAST_RUN = {}


@functools.lru_cache(maxsize=1)
def build_nc() -> bass.Bass:
    nc = bacc.Bacc(
        "TRN2",
        target_bir_lowering=False,
        debug=False,
        num_devices=NCORES,
    )
    ctxTh = nc.declare_dram_parameter("ctxTh", [D, NR], BF16, isOutput=False)
    ctxTl = nc.declare_dram_parameter("ctxTl", [D, NR], BF16, isOutput=False)
    wkTh = nc.declare_dram_parameter("wkTh", [S, D, D], BF16, isOutput=False)
    wkTl = nc.declare_dram_parameter("wkTl", [S, D, D], BF16, isOutput=False)
    wkbH = nc.declare_dram_parameter("wkbH", [1, S, D], BF16, isOutput=False)
    wkbL = nc.declare_dram_parameter("wkbL", [1, S, D], BF16, isOutput=False)
    ench = nc.declare_dram_parameter("ench", [B * G * G, D], FP16, isOutput=False)
    idx = nc.declare_dram_parameter(
        "idx", [128, IDX_TOT // 16], mybir.dt.int16, isOutput=False
    )
    out = nc.declare_dram_parameter("out", [1, 2], F32, isOutput=True)

    Alu = mybir.AluOpType
    Act = mybir.ActivationFunctionType
    Ax = mybir.AxisListType

    with tile.TileContext(nc) as tc:
        with (
            tc.tile_pool(name="const", bufs=1) as constp,
            tc.tile_pool(name="wk", bufs=2) as wkp,
            tc.tile_pool(name="pred", bufs=NSG) as predp,
            tc.tile_pool(name="stage", bufs=2) as stagep,
            tc.tile_pool(name="gath", bufs=3) as gathp,
            tc.tile_pool(name="scr", bufs=1) as scrp,
            tc.tile_pool(name="dots", bufs=2) as dotsp,
            tc.tile_pool(name="small", bufs=4) as smallp,
            tc.tile_pool(name="acc", bufs=1) as accp,
            tc.tile_pool(name="psum", bufs=3, space="PSUM") as psump,
            tc.tile_pool(name="psumf", bufs=1, space="PSUM") as psumfp,
        ):
            # ---- constants / accumulators ----
            idx_sb = constp.tile([128, IDX_TOT // 16], mybir.dt.int16, tag="idx")
            nc.sync.dma_start(idx_sb[:, :], idx[:, :])
            ones_sb = constp.tile([128, 1], F32, tag="ones")
            nc.vector.memset(ones_sb[:, :], 1.0)
            onesb16 = constp.tile([1, 128], BF16, tag="onesb16")
            nc.vector.memset(onesb16[:, :], 1.0)
            acc2 = accp.tile([128, 2], F32, tag="acc2")
            nc.vector.memset(acc2[:, :], 0.0)
            wkbh_sb = constp.tile([1, S, D], BF16, tag="wkbh")
            wkbl_sb = constp.tile([1, S, D], BF16, tag="wkbl")
            nc.sync.dma_start(wkbh_sb[:, :, :], wkbH[:, :, :])
            nc.sync.dma_start(wkbl_sb[:, :, :], wkbL[:, :, :])

            # resident bf16 hi/lo ctx^T: [128 d_in, 10 d_out, NR rows]
            ctxh_sb = constp.tile([128, 10, NR], BF16, tag="ctxh")
            ctxl_sb = constp.tile([128, 10, NR], BF16, tag="ctxl")
            nc.sync.dma_start(
                ctxh_sb[:, :, :], ctxTh[:, :].rearrange("(do di) r -> di do r", di=128)
            )
            nc.sync.dma_start(
                ctxl_sb[:, :, :], ctxTl[:, :].rearrange("(do di) r -> di do r", di=128)
            )

            pred_tiles = [
                predp.tile([128, D], FP16, tag="pred", name=f"pred{i}")
                for i in range(NSG)
            ]
            # rows 96..127 of the last supergroup are never written by the
            # repack; zero them so phase-2 reads are defined.
            nc.vector.memset(pred_tiles[8][96:128, :], 0.0)

            # ---- phase 1: pred = ctx @ Wk^T + b (3-pass bf16 hi/lo) ----
            for s in range(S):
                wkh_r = wkTh[s, :, :].rearrange("(do di) e -> di do e", di=128)
                wkl_r = wkTl[s, :, :].rearrange("(do di) e -> di do e", di=128)
                for eh in range(2):
                    e0 = eh * E_HALF
                    wkh_t = wkp.tile([128, 10, E_HALF], BF16, tag="wkh")
                    wkl_t = wkp.tile([128, 10, E_HALF], BF16, tag="wkl")
                    nc.sync.dma_start(wkh_t[:, :, :], wkh_r[:, :, e0 : e0 + E_HALF])
                    nc.sync.dma_start(wkl_t[:, :, :], wkl_r[:, :, e0 : e0 + E_HALF])
                    for t in range(T_S[s]):
                        M = min(128, NS[s] - 128 * t)
                        roff = SOFF[s] + 128 * t
                        ch = ctxh_sb[:, :, roff : roff + M]
                        cl = ctxl_sb[:, :, roff : roff + M]
                        ps = psump.tile([128, E_HALF], F32, tag="ps")
                        for eoff, ew in ((0, 512), (512, 128)):
                            for d in range(10):
                                nc.tensor.matmul(
                                    ps[:M, eoff : eoff + ew],
                                    lhsT=ch[:, d, :],
                                    rhs=wkh_t[:, d, eoff : eoff + ew],
                                    start=(d == 0), stop=False,
                                )
                                nc.tensor.matmul(
                                    ps[:M, eoff : eoff + ew],
                                    lhsT=ch[:, d, :],
                                    rhs=wkl_t[:, d, eoff : eoff + ew],
                                    start=False, stop=False,
                                )
                                nc.tensor.matmul(
                                    ps[:M, eoff : eoff + ew],
                                    lhsT=cl[:, d, :],
                                    rhs=wkh_t[:, d, eoff : eoff + ew],
                                    start=False, stop=False,
                                )
                            # bias via K=1 matmuls (hi + lo)
                            nc.tensor.matmul(
                                ps[:M, eoff : eoff + ew],
                                lhsT=onesb16[0:1, :M],
                                rhs=wkbh_sb[0:1, s, e0 + eoff : e0 + eoff + ew],
                                start=False, stop=False,
                            )
                            nc.tensor.matmul(
                                ps[:M, eoff : eoff + ew],
                                lhsT=onesb16[0:1, :M],
                                rhs=wkbl_sb[0:1, s, e0 + eoff : e0 + eoff + ew],
                                start=False, stop=True,
                            )
                        # PSUM -> fp16 staging on the scalar engine (keeps DVE free)
                        stg = stagep.tile([128, E_HALF], FP16, tag="stg")
                        nc.scalar.copy(stg[:M, :], ps[:M, :])
                        # repack into dense 128-row supergroup tiles
                        k, p0 = divmod(roff, 128)
                        n1 = min(M, 128 - p0)
                        nc.sync.dma_start(
                            pred_tiles[k][p0 : p0 + n1, e0 : e0 + E_HALF],
                            stg[0:n1, :],
                        )
                        if M > n1:
                            nc.sync.dma_start(
                                pred_tiles[k + 1][0 : M - n1, e0 : e0 + E_HALF],
                                stg[n1:M, :],
                            )

            # ---- phase 2: gather fp16 targets, fused dots, CE ----
            ench_ap = ench[:, :]
            for sg in range(NSG):
                dots_t = dotsp.tile([128, NDOT], F32, tag="dots")
                for goff, w in GCHUNKS:
                    gt = gathp.tile([128, 4, D], FP16, tag="gt")
                    pos0 = sg * IDX_PER_SG + goff * 128
                    nidx = w * 128
                    nc.gpsimd.dma_gather(
                        gt[:, :w, :],
                        ench_ap,
                        idx_sb[:, pos0 // 16 : (pos0 + nidx) // 16],
                        nidx,
                        nidx,
                        D,
                    )
                    for j in range(w):
                        scr = scrp.tile([128, D], F32, tag="scr")
                        g = goff + j
                        # fused dot: out = (gt * 1.0) * pred, accum = sum(out)
                        nc.vector.scalar_tensor_tensor(
                            scr[:, :],
                            gt[:, j, :],
                            1.0,
                            pred_tiles[sg][:, :],
                            op0=Alu.mult,
                            op1=Alu.mult,
                            accum_out=dots_t[:, g : g + 1],
                        )
                # softmax-CE on the 17 logits; logit 0 is the positive
                negm = smallp.tile([128, 1], F32, tag="negm")
                nc.vector.tensor_reduce(
                    negm[:, :], dots_t[:, :], Ax.X, Alu.max, negate=True
                )
                e_t = scrp.tile([128, NDOT], F32, tag="et")
                ssum = smallp.tile([128, 1], F32, tag="ssum")
                nc.scalar.activation(
                    e_t[:, :],
                    dots_t[:, :],
                    Act.Exp,
                    bias=negm[:, 0:1],
                    scale=1.0,
                    accum_out=ssum[:, :],
                )
                lns = smallp.tile([128, 1], F32, tag="lns")
                nc.scalar.activation(lns[:, :], ssum[:, :], Act.Ln)
                # loss = ln(sum) + m - pos  (negm = -m)
                tmp = smallp.tile([128, 1], F32, tag="tmp")
                nc.vector.tensor_tensor(tmp[:, :], lns[:, :], negm[:, :], Alu.subtract)
                lossr = smallp.tile([128, 1], F32, tag="lossr")
                nc.vector.tensor_tensor(
                    lossr[:, :], tmp[:, :], dots_t[:, 0:1], Alu.subtract
                )
                maxneg = smallp.tile([128, 1], F32, tag="maxneg")
                nc.vector.tensor_reduce(
                    maxneg[:, :], dots_t[:, 1:NDOT], Ax.X, Alu.max
                )
                corr = smallp.tile([128, 1], F32, tag="corr")
                nc.vector.tensor_tensor(
                    corr[:, :], dots_t[:, 0:1], maxneg[:, :], Alu.is_ge
                )
                v = SG_VALID[sg]
                nc.vector.tensor_tensor(
                    acc2[:v, 0:1], acc2[:v, 0:1], lossr[:v, :], Alu.add
                )
                nc.vector.tensor_tensor(
                    acc2[:v, 1:2], acc2[:v, 1:2], corr[:v, :], Alu.add
                )

            # ---- final partition reduce: [128,2] -> [1,2] ----
            psf = psumfp.tile([1, 2], F32, tag="psf")
            nc.tensor.matmul(
                psf[:, :], lhsT=ones_sb[:, 0:1], rhs=acc2[:, :], start=True, stop=True
            )
            outsb = smallp.tile([1, 2], F32, tag="outsb")
            nc.vector.tensor_copy(outsb[:, :], psf[:, :])
            nc.sync.dma_start(out[:, :], outsb[:, :])

    nc.compile()
    return nc


def _row_targets(core: int, neg_idx: np.ndarray) -> np.ndarray:
    """[NR, 17] int array: flat enc index of positive + 16 negatives per row."""
    tg = np.zeros((NR, NDOT), np.int64)
    ri = 0
    for s in range(S):
        rows = 6 - s
        for b in range(BSH):
            bg = core * BSH + b
            for r in range(rows):
                for c7 in range(G):
                    tg[ri, 0] = bg * G * G + (s + 1 + r) * G + c7
                    tg[ri, 1:] = neg_idx[bg, s, r, c7]
                    ri += 1
    assert ri == NR
    return tg


def _build_idx(core: int, neg_idx: np.ndarray) -> np.ndarray:
    """int16 [128, IDX_TOT//16] gather-index tensor in SWDGE wrap layout."""
    tg = _row_targets(core, neg_idx)
    tg_pad = np.zeros((NSG * 128, NDOT), np.int64)
    tg_pad[:NR] = tg
    # list position sg*2176 + g*128 + p  ->  target of (row sg*128+p, dot g)
    lst = tg_pad.reshape(NSG, 128, NDOT).transpose(0, 2, 1).reshape(-1)
    arr = lst.astype(np.int16).reshape(-1, 16).T  # [16, IDX_TOT//16]
    return np.ascontiguousarray(np.tile(arr, (8, 1)))  # [128, ...]


def _split_bf16(x: np.ndarray):
    h = x.astype(ml_dtypes.bfloat16)
    l = (x - h.astype(np.float32)).astype(ml_dtypes.bfloat16)
    return h, l


def _prep_in_maps(contexts, encodings, Wk_w, Wk_b, neg_idx):
    contexts = np.ascontiguousarray(np.asarray(contexts, np.float32))
    encodings = np.ascontiguousarray(np.asarray(encodings, np.float32))
    Wk_w = np.ascontiguousarray(np.asarray(Wk_w, np.float32))
    Wk_b = np.ascontiguousarray(np.asarray(Wk_b, np.float32))
    neg_idx = np.asarray(neg_idx)

    ench = np.ascontiguousarray(
        encodings.reshape(B * G * G, D).astype(np.float16)
    )
    wkT = Wk_w.transpose(0, 2, 1)  # [S, d, e]
    wkTh, wkTl = _split_bf16(wkT)
    wkTh = np.ascontiguousarray(wkTh)
    wkTl = np.ascontiguousarray(wkTl)
    wkbH, wkbL = _split_bf16(Wk_b[None, :, :])
    wkbH = np.ascontiguousarray(wkbH)
    wkbL = np.ascontiguousarray(wkbL)

    in_maps = []
    for c in range(NCORES):
        bs = slice(c * BSH, (c + 1) * BSH)
        ctx_rows = np.concatenate(
            [contexts[bs, : 6 - s].reshape(-1, D) for s in range(S)], axis=0
        )
        ctxT = ctx_rows.T  # [d, NR]
        ctxTh, ctxTl = _split_bf16(ctxT)
        in_maps.append(
            {
                "ctxTh": np.ascontiguousarray(ctxTh),
                "ctxTl": np.ascontiguousarray(ctxTl),
                "wkTh": wkTh,
                "wkTl": wkTl,
                "wkbH": wkbH,
                "wkbL": wkbL,
                "ench": ench,
                "idx": _build_idx(c, neg_idx),
            }
        )
    return in_maps


def kernel(contexts, encodings, Wk_w, Wk_b, neg_idx, _trace=False):
    in_maps = _prep_in_maps(contexts, encodings, Wk_w, Wk_b, neg_idx)
    nc = build_nc()
    res = run_bass_kernel_spmd(nc, in_maps, list(range(NCORES)), trace=_trace)
    LAST_RUN["exec_time_ns"] = res.exec_time_ns
    LAST_RUN["results"] = res.results
    loss = np.float32(0.0)
    corr = np.float32(0.0)
    for o in res.results:
        loss += np.float32(o["out"][0, 0])
        corr += np.float32(o["out"][0, 1])
    return (
        np.float32(loss / np.float32(N_PREDS)),
        np.float32(corr / np.float32(N_PREDS)),
    )



# revision 8
# speedup vs baseline: 1.2420x; 1.2420x over previous
"""Trainium2 Bass kernel for the CPC loss problem (nn_CPC_85117661872355).

Strategy (data-parallel over batch B across 8 cores):
  - Each core handles 8 of the 64 batch elements: 1120 prediction rows.
  - pred = ctx @ Wk[s]^T + b on the PE as a SINGLE-pass fp16 matmul
    (fp32 PSUM accumulate).  The 2e-2 rel-err gate leaves plenty of room:
    fp16 inputs give ~2e-4 logit noise vs typical top-2 gaps ~0.09.
  - All 17 logits per row (1 positive + 16 negatives) are dot products
    pred_row . enc_flat[idx].  Target vectors are fetched with SWDGE
    dma_gather from an fp16 copy of the encoding table and the dots are
    computed with fused DVE scalar_tensor_tensor ops in all-16-bit mode
    (2 elem/cycle).  Gathering the positive through the same path keeps
    bitwise ties when a negative index collides with the positive,
    matching jnp.argmax's first-index tie-break.
  - Per-supergroup softmax stats (exp-sum, pos-minus-max, correct) are
    collected into [128,9] accumulators; a single deferred Ln pass at the
    end avoids per-supergroup activation table swaps.
  - Phase 1 and phase 2 are interleaved in emission order so the PE
    matmuls, SWDGE gathers, DVE dots and ACT ops all pipeline.
  - Per-core (loss_sum, correct_sum) reduced over partitions with a
    ones-matmul and DMA'd out as [1,2]; host sums the 8 partial pairs.
"""

import functools

import numpy as np

import concourse.bass as bass
import concourse.mybir as mybir
import concourse.tile as tile
from concourse import bacc
from concourse.bass_utils import run_bass_kernel_spmd

F32 = mybir.dt.float32
FP16 = mybir.dt.float16

B, G, D = 64, 7, 1280
S, NEG = 5, 16
NCORES = 8
BSH = B // NCORES  # 8
NS = [BSH * (6 - s) * G for s in range(S)]  # [336, 280, 224, 168, 112]
SOFF = [0]
for n in NS:
    SOFF.append(SOFF[-1] + n)
NR = SOFF[-1]  # 1120 rows per core
NSG = 9  # supergroups of 128 rows
NDOT = 17  # 1 positive + 16 negatives
GCHUNKS = [(0, 8), (8, 8), (16, 1)]  # gather (goff, width) per supergroup
IDX_PER_SG = NDOT * 128  # 2176
IDX_TOT = NSG * IDX_PER_SG  # 19584
N_PREDS = B * G * 20  # 8960

# Results of the last device run (for test harness introspection)
LAST_RUN = {}


@functools.lru_cache(maxsize=1)
def build_nc() -> bass.Bass:
    nc = bacc.Bacc(
        "TRN2",
        target_bir_lowering=False,
        debug=False,
        num_devices=NCORES,
    )
    ctxT = nc.declare_dram_parameter("ctxT", [D, NR], FP16, isOutput=False)
    wkT = nc.declare_dram_parameter("wkT", [S, D, D], FP16, isOutput=False)
    wkb = nc.declare_dram_parameter("wkb", [1, S, D], FP16, isOutput=False)
    ench = nc.declare_dram_parameter("ench", [B * G * G, D], FP16, isOutput=False)
    idx = nc.declare_dram_parameter(
        "idx", [128, IDX_TOT // 16], mybir.dt.int16, isOutput=False
    )
    out = nc.declare_dram_parameter("out", [1, 2], F32, isOutput=True)

    Alu = mybir.AluOpType
    Act = mybir.ActivationFunctionType
    Ax = mybir.AxisListType

    # tiles per s (row-tiles of up to 128 rows, in global row order)
    T_S = [(NS[s] + 127) // 128 for s in range(S)]  # [3, 3, 2, 2, 1]
    # supergroup sg is fully computed after phase-1 tile (s, t) when all rows
    # < (sg+1)*128 are covered (sg 8 is only 96 valid rows).
    SG_END = [128 * sg + (96 if sg == 8 else 128) for sg in range(NSG)]

    with tile.TileContext(nc) as tc:
        with (
            tc.tile_pool(name="const", bufs=1) as constp,
            tc.tile_pool(name="wk", bufs=2) as wkp,
            tc.tile_pool(name="pred", bufs=NSG) as predp,
            tc.tile_pool(name="stage", bufs=2) as stagep,
            tc.tile_pool(name="gath", bufs=3) as gathp,
            tc.tile_pool(name="scr", bufs=1) as scrp,
            tc.tile_pool(name="dots", bufs=2) as dotsp,
            tc.tile_pool(name="small", bufs=4) as smallp,
            tc.tile_pool(name="acc", bufs=1) as accp,
            tc.tile_pool(name="psum", bufs=2, space="PSUM") as psump,
            tc.tile_pool(name="psumf", bufs=1, space="PSUM") as psumfp,
        ):
            # ---- constants / accumulators ----
            idx_sb = constp.tile([128, IDX_TOT // 16], mybir.dt.int16, tag="idx")
            nc.sync.dma_start(idx_sb[:, :], idx[:, :])
            ones_sb = constp.tile([128, 1], F32, tag="ones")
            nc.vector.memset(ones_sb[:, :], 1.0)
            ones16 = constp.tile([1, 128], FP16, tag="ones16")
            nc.vector.memset(ones16[:, :], 1.0)
            wkb_sb = constp.tile([1, S, D], FP16, tag="wkb")
            nc.sync.dma_start(wkb_sb[:, :, :], wkb[:, :, :])

            # resident fp16 ctx^T: [128 d_in, 10 d_blk, NR rows] (2 chunk loads)
            ctx_sb = constp.tile([128, 10, NR], FP16, tag="ctx")
            ctx_r = ctxT[:, :].rearrange("(do di) r -> di do r", di=128)
            nc.sync.dma_start(ctx_sb[:, :, 0:560], ctx_r[:, :, 0:560])
            nc.sync.dma_start(ctx_sb[:, :, 560:NR], ctx_r[:, :, 560:NR])

            pred_tiles = [
                predp.tile([128, D], FP16, tag="pred", name=f"pred{i}")
                for i in range(NSG)
            ]
            # rows 96..127 of the last supergroup are never written by the
            # repack; zero them so phase-2 reads are defined (keeps exp finite).
            nc.vector.memset(pred_tiles[8][96:128, :], 0.0)

            # per-sg CE stats: ss9 = sum(exp(dots-m)); pm9 = pos - m;
            # lc18 cols 0..8 become the per-sg row losses, cols 9..17 the
            # per-sg correct flags.
            ss9 = accp.tile([128, NSG], F32, tag="ss9")
            pm9 = accp.tile([128, NSG], F32, tag="pm9")
            lc18 = accp.tile([128, 2 * NSG], F32, tag="lc18")

            # ---- gathers: issued early/JIT, throttled by the pool ring ----
            ench_ap = ench[:, :]
            gt_tiles = {}
            gcursor = [0]

            def emit_gathers(upto):
                while gcursor[0] < min(upto, NSG * len(GCHUNKS)):
                    k = gcursor[0]
                    sg, ci = divmod(k, len(GCHUNKS))
                    goff, w = GCHUNKS[ci]
                    gt = gathp.tile([128, 8, D], FP16, tag="gt")
                    pos0 = sg * IDX_PER_SG + goff * 128
                    nidx = w * 128
                    nc.gpsimd.dma_gather(
                        gt[:, :w, :],
                        ench_ap,
                        idx_sb[:, pos0 // 16 : (pos0 + nidx) // 16],
                        nidx,
                        nidx,
                        D,
                    )
                    gt_tiles[(sg, ci)] = gt
                    gcursor[0] += 1

            emit_gathers(3)

            def emit_phase2(sg):
                dots_t = dotsp.tile([128, NDOT], F32, tag="dots")
                for ci, (goff, w) in enumerate(GCHUNKS):
                    gt = gt_tiles.pop((sg, ci))
                    for j in range(w):
                        scr = scrp.tile([128, D], FP16, tag="scr")
                        g = goff + j
                        # fused dot: out = (gt * 1.0) * pred, accum = sum(out)
                        nc.vector.scalar_tensor_tensor(
                            scr[:, :],
                            gt[:, j, :],
                            1.0,
                            pred_tiles[sg][:, :],
                            op0=Alu.mult,
                            op1=Alu.mult,
                            accum_out=dots_t[:, g : g + 1],
                        )
                # stats for deferred CE; logit 0 is the positive
                negm = smallp.tile([128, 1], F32, tag="negm")
                nc.vector.tensor_reduce(
                    negm[:, :], dots_t[:, :], Ax.X, Alu.max, negate=True
                )
                e_t = smallp.tile([128, NDOT], F32, tag="et")
                nc.scalar.activation(
                    e_t[:, :],
                    dots_t[:, :],
                    Act.Exp,
                    bias=negm[:, 0:1],
                    scale=1.0,
                    accum_out=ss9[:, sg : sg + 1],
                )
                nc.vector.tensor_tensor(
                    pm9[:, sg : sg + 1], dots_t[:, 0:1], negm[:, :], Alu.add
                )
                maxneg = smallp.tile([128, 1], F32, tag="maxneg")
                nc.vector.tensor_reduce(
                    maxneg[:, :], dots_t[:, 1:NDOT], Ax.X, Alu.max
                )
                nc.vector.tensor_tensor(
                    lc18[:, NSG + sg : NSG + sg + 1],
                    dots_t[:, 0:1],
                    maxneg[:, :],
                    Alu.is_ge,
                )
                emit_gathers(len(GCHUNKS) * (sg + 1) + 3)

            # ---- phase 1: pred = ctx @ Wk^T + b (single-pass fp16) ----
            ECH = [(0, 512), (512, 512), (1024, 256)]
            sg_done = 0
            rows_done = 0
            for s in range(S):
                wk_t = wkp.tile([128, 10, D], FP16, tag="wk")
                wk_r = wkT[s, :, :].rearrange("(do di) e -> di do e", di=128)
                for dch in range(2):
                    nc.sync.dma_start(
                        wk_t[:, 5 * dch : 5 * dch + 5, :],
                        wk_r[:, 5 * dch : 5 * dch + 5, :],
                    )
                for t in range(T_S[s]):
                    M = min(128, NS[s] - 128 * t)
                    roff = SOFF[s] + 128 * t
                    # 1536 wide = 3 PSUM banks exactly; each 512-col chunk is
                    # bank-aligned (one matmul output must stay in one bank).
                    ps = psump.tile([128, 1536], F32, tag="ps")
                    for d in range(10):
                        for eoff, ew in ECH:
                            nc.tensor.matmul(
                                ps[:M, eoff : eoff + ew],
                                lhsT=ctx_sb[:, d, roff : roff + M],
                                rhs=wk_t[:, d, eoff : eoff + ew],
                                start=(d == 0),
                                stop=False,
                            )
                    for eoff, ew in ECH:
                        nc.tensor.matmul(
                            ps[:M, eoff : eoff + ew],
                            lhsT=ones16[0:1, :M],
                            rhs=wkb_sb[0:1, s, eoff : eoff + ew],
                            start=False,
                            stop=True,
                        )
                    # PSUM -> fp16 staging on the scalar engine
                    stg = stagep.tile([128, D], FP16, tag="stg")
                    nc.scalar.copy(stg[:M, :], ps[:M, 0:D])
                    # repack into dense 128-row supergroup tiles
                    k, p0 = divmod(roff, 128)
                    n1 = min(M, 128 - p0)
                    nc.sync.dma_start(
                        pred_tiles[k][p0 : p0 + n1, :], stg[0:n1, :]
                    )
                    if M > n1:
                        nc.sync.dma_start(
                            pred_tiles[k + 1][0 : M - n1, :], stg[n1:M, :]
                        )
                    rows_done = roff + M
                    while sg_done < NSG and rows_done >= SG_END[sg_done]:
                        emit_phase2(sg_done)
                        sg_done += 1
            assert sg_done == NSG

            # ---- deferred CE finale ----
            # loss_r = ln(ss) - (pos - m); lc18 cols 0..8 <- that, 9..17 = corr
            ln9 = smallp.tile([128, NSG], F32, tag="ln9")
            nc.scalar.activation(ln9[:, :], ss9[:, :], Act.Ln)
            nc.vector.tensor_tensor(
                lc18[:, 0:NSG], ln9[:, :], pm9[:, :], Alu.subtract
            )
            # zero the 32 invalid rows of supergroup 8
            nc.vector.memset(lc18[96:128, 8:9], 0.0)
            nc.vector.memset(lc18[96:128, NSG + 8 : NSG + 9], 0.0)

            # ---- final partition reduce: [128,18] -> [1,18] -> [1,2] ----
            psf = psumfp.tile([1, 2 * NSG], F32, tag="psf")
            nc.tensor.matmul(
                psf[:, :], lhsT=ones_sb[:, 0:1], rhs=lc18[:, :], start=True,
                stop=True,
            )
            sum18 = smallp.tile([1, 2 * NSG], F32, tag="sum18")
            nc.vector.tensor_copy(sum18[:, :], psf[:, :])
            outsb = smallp.tile([1, 2], F32, tag="outsb")
            nc.vector.tensor_reduce(
                outsb[:, 0:2],
                sum18[:, :].rearrange("p (a b) -> p a b", a=2),
                Ax.X,
                Alu.add,
            )
            nc.sync.dma_start(out[:, :], outsb[:, :])

    nc.compile()
    return nc


def _row_targets(core: int, neg_idx: np.ndarray) -> np.ndarray:
    """[NR, 17] int array: flat enc index of positive + 16 negatives per row."""
    tg = np.zeros((NR, NDOT), np.int64)
    ri = 0
    for s in range(S):
        rows = 6 - s
        for b in range(BSH):
            bg = core * BSH + b
            for r in range(rows):
                for c7 in range(G):
                    tg[ri, 0] = bg * G * G + (s + 1 + r) * G + c7
                    tg[ri, 1:] = neg_idx[bg, s, r, c7]
                    ri += 1
    assert ri == NR
    return tg


def _build_idx(core: int, neg_idx: np.ndarray) -> np.ndarray:
    """int16 [128, IDX_TOT//16] gather-index tensor in SWDGE wrap layout."""
    tg = _row_targets(core, neg_idx)
    tg_pad = np.zeros((NSG * 128, NDOT), np.int64)
    tg_pad[:NR] = tg
    # list position sg*2176 + g*128 + p  ->  target of (row sg*128+p, dot g)
    lst = tg_pad.reshape(NSG, 128, NDOT).transpose(0, 2, 1).reshape(-1)
    arr = lst.astype(np.int16).reshape(-1, 16).T  # [16, IDX_TOT//16]
    return np.ascontiguousarray(np.tile(arr, (8, 1)))  # [128, ...]


def _prep_in_maps(contexts, encodings, Wk_w, Wk_b, neg_idx):
    contexts = np.ascontiguousarray(np.asarray(contexts, np.float32))
    encodings = np.ascontiguousarray(np.asarray(encodings, np.float32))
    Wk_w = np.ascontiguousarray(np.asarray(Wk_w, np.float32))
    Wk_b = np.ascontiguousarray(np.asarray(Wk_b, np.float32))
    neg_idx = np.asarray(neg_idx)

    ench = np.ascontiguousarray(
        encodings.reshape(B * G * G, D).astype(np.float16)
    )
    wkT = np.ascontiguousarray(Wk_w.transpose(0, 2, 1).astype(np.float16))
    wkb = np.ascontiguousarray(Wk_b[None, :, :].astype(np.float16))

    in_maps = []
    for c in range(NCORES):
        bs = slice(c * BSH, (c + 1) * BSH)
        ctx_rows = np.concatenate(
            [contexts[bs, : 6 - s].reshape(-1, D) for s in range(S)], axis=0
        )
        in_maps.append(
            {
                "ctxT": np.ascontiguousarray(ctx_rows.T.astype(np.float16)),
                "wkT": wkT,
                "wkb": wkb,
                "ench": ench,
                "idx": _build_idx(c, neg_idx),
            }
        )
    return in_maps


def kernel(contexts, encodings, Wk_w, Wk_b, neg_idx, _trace=False):
    in_maps = _prep_in_maps(contexts, encodings, Wk_w, Wk_b, neg_idx)
    nc = build_nc()
    res = run_bass_kernel_spmd(nc, in_maps, list(range(NCORES)), trace=_trace)
    LAST_RUN["exec_time_ns"] = res.exec_time_ns
    LAST_RUN["results"] = res.results
    loss = np.float32(0.0)
    corr = np.float32(0.0)
    for o in res.results:
        loss += np.float32(o["out"][0, 0])
        corr += np.float32(o["out"][0, 1])
    return (
        np.float32(loss / np.float32(N_PREDS)),
        np.float32(corr / np.float32(N_PREDS)),
    )


# revision 12
# speedup vs baseline: 1.5022x; 1.2094x over previous
"""Trainium2 Bass kernel for the CPC loss problem (nn_CPC_85117661872355).

Strategy (data-parallel over batch B across 8 cores):
  - Each core handles 8 of the 64 batch elements: 1120 prediction rows.
  - Phase 1 computes pred TRANSPOSED: predT[e, r] = sum_d Wk[s][e,d] ctx[r,d]
    + b[e], as a single-pass fp16 matmul (Wk blocks stationary, ctx^T rows
    moving, fp32 PSUM accumulate).  E lands on partitions, so the ragged
    per-s row groups go on the free axis and no repack is needed.
  - All 17 logits per row (1 positive + 16 negatives) are dot products
    pred_row . enc_flat[idx], contracted over E.  Target vectors are fetched
    with SWDGE dma_gather(transpose=True) from an fp16 copy of the encoding
    table, which lands them E-on-partitions: gtT[e_sub, e_blk, j].  Each
    128-dot group is then a 10-matmul PE accumulation
    out[r, j] = sum_e predT[e, r] gtT[e, j] whose DIAGONAL holds the dots;
    a fused DVE scalar_tensor_tensor against a host-supplied identity
    extracts diag+accumulates in one [128,128] op.  This moves the 25M
    multiply-adds of the dot products from DVE (1 elem/cycle) to the PE.
  - Gathering the positive through the same path keeps bitwise ties when a
    negative index collides with the positive (jnp.argmax first-index
    tie-break).
  - Dots tiles for all 9 supergroups are kept; softmax-CE runs once at the
    end (one Exp table load, one Ln), accumulating loss/correct per
    partition; a ones-matmul reduces to [1,2] per core; host sums cores.
"""

import functools

import numpy as np

import concourse.bass as bass
import concourse.mybir as mybir
import concourse.tile as tile
from concourse import bacc
from concourse.bass_utils import run_bass_kernel_spmd

F32 = mybir.dt.float32
FP16 = mybir.dt.float16

B, G, D = 64, 7, 1280
S, NEG = 5, 16
NCORES = 8
BSH = B // NCORES  # 8
NS = [BSH * (6 - s) * G for s in range(S)]  # [336, 280, 224, 168, 112]
SOFF = [0]
for n in NS:
    SOFF.append(SOFF[-1] + n)
NR = SOFF[-1]  # 1120 rows per core
NSG = 9  # supergroups of 128 rows
NDOT = 17  # 1 positive + 16 negatives
GCHUNKS = [(0, 6), (6, 6), (12, 5)]  # gather (goff, width) per supergroup
IDX_PER_SG = NDOT * 128  # 2176
IDX_TOT = NSG * IDX_PER_SG  # 19584
N_PREDS = B * G * 20  # 8960

# Results of the last device run (for test harness introspection)
LAST_RUN = {}


@functools.lru_cache(maxsize=1)
def build_nc() -> bass.Bass:
    nc = bacc.Bacc(
        "TRN2",
        target_bir_lowering=False,
        debug=False,
        num_devices=NCORES,
    )
    # ctxT: [d, r] with d split [128 d_sub, 10 d_blk]
    ctxT = nc.declare_dram_parameter("ctxT", [D, NR], FP16, isOutput=False)
    # wk5: [128 d_in_sub, S, 10 d_out_blk(e), 10 d_in_blk, 128 e_sub]
    # element [di, s, eo, do, ei] = Wk_w[s, eo*128+ei, do*128+di]
    wk5 = nc.declare_dram_parameter("wk5", [128, S, 10, 10, 128], FP16,
                                    isOutput=False)
    wkb = nc.declare_dram_parameter("wkb", [1, S, 10, 128], FP16,
                                    isOutput=False)
    ench = nc.declare_dram_parameter("ench", [B * G * G, D], FP16,
                                     isOutput=False)
    ident = nc.declare_dram_parameter("ident", [128, 128], FP16,
                                      isOutput=False)
    idx = nc.declare_dram_parameter(
        "idx", [128, IDX_TOT // 16], mybir.dt.int16, isOutput=False
    )
    out = nc.declare_dram_parameter("out", [1, 2], F32, isOutput=True)

    Alu = mybir.AluOpType
    Act = mybir.ActivationFunctionType
    Ax = mybir.AxisListType

    # supergroups fully covered after each s finishes phase 1
    SG_AFTER_S = [[0, 1], [2, 3], [4, 5], [6], [7, 8]]
    SG_M = [128] * 8 + [96]  # valid rows per supergroup

    with tile.TileContext(nc) as tc:
        with (
            tc.tile_pool(name="const", bufs=1) as constp,
            tc.tile_pool(name="wk", bufs=2) as wkp,
            tc.tile_pool(name="gath", bufs=3) as gathp,
            tc.tile_pool(name="gath1", bufs=2) as gath1p,
            tc.tile_pool(name="dots", bufs=NSG) as dotsp,
            tc.tile_pool(name="small", bufs=4) as smallp,
            tc.tile_pool(name="acc", bufs=1) as accp,
            tc.tile_pool(name="psA", bufs=2, space="PSUM") as psAp,
            tc.tile_pool(name="psD", bufs=3, space="PSUM") as psDp,
            tc.tile_pool(name="psF", bufs=1, space="PSUM") as psFp,
        ):
            # ---- constants ----
            idx_sb = constp.tile([128, IDX_TOT // 16], mybir.dt.int16,
                                 tag="idx")
            nc.sync.dma_start(idx_sb[:, :], idx[:, :])
            ident_sb = constp.tile([128, 128], FP16, tag="ident")
            nc.sync.dma_start(ident_sb[:, :], ident[:, :])
            ones_sb = constp.tile([128, 1], F32, tag="ones")
            nc.vector.memset(ones_sb[:, :], 1.0)
            ones16 = constp.tile([1, 512], FP16, tag="ones16")
            nc.vector.memset(ones16[:, :], 1.0)
            wkb_sb = constp.tile([1, S, 10, 128], FP16, tag="wkb")
            nc.sync.dma_start(wkb_sb[:, :, :, :], wkb[:, :, :, :])

            # resident fp16 ctx^T: [128 d_sub, 10 d_blk, NR rows]
            ctx_sb = constp.tile([128, 10, NR], FP16, tag="ctx")
            ctx_r = ctxT[:, :].rearrange("(do di) r -> di do r", di=128)
            nc.sync.dma_start(ctx_sb[:, :, 0:560], ctx_r[:, :, 0:560])
            nc.sync.dma_start(ctx_sb[:, :, 560:NR], ctx_r[:, :, 560:NR])

            # predT resident: [128 e_sub, 10 e_blk, NR rows] fp16
            predT = constp.tile([128, 10, NR], FP16, tag="predT")

            # CE stat accumulators
            ss9 = accp.tile([128, NSG], F32, tag="ss9")
            pm9 = accp.tile([128, NSG], F32, tag="pm9")
            lc18 = accp.tile([128, 2 * NSG], F32, tag="lc18")
            scr128 = accp.tile([128, 128], FP16, tag="scr128")

            # ---- gathers (transposed): issued with small lookahead ----
            ench_ap = ench[:, :]
            gt_tiles = {}
            gcursor = [0]

            def emit_gathers(upto):
                while gcursor[0] < min(upto, NSG * len(GCHUNKS)):
                    k = gcursor[0]
                    sg, ci = divmod(k, len(GCHUNKS))
                    goff, w = GCHUNKS[ci]
                    nidx = w * 128
                    pool = gathp if w == 6 else gath1p
                    gt = pool.tile([128, 10, nidx], FP16, tag=f"gt{w}")
                    pos0 = sg * IDX_PER_SG + goff * 128
                    nc.gpsimd.dma_gather(
                        gt[:, :, :],
                        ench_ap,
                        idx_sb[:, pos0 // 16 : (pos0 + nidx) // 16],
                        nidx,
                        nidx,
                        D,
                        transpose=True,
                    )
                    gt_tiles[(sg, ci)] = gt
                    gcursor[0] += 1

            emit_gathers(3)

            dots_tiles = [
                dotsp.tile([128, NDOT], F32, tag="dots", name=f"dots{i}")
                for i in range(NSG)
            ]
            # rows 96.. of supergroup 8 never get extracts; keep them finite
            nc.vector.memset(dots_tiles[8][96:128, :], 0.0)

            def emit_phase2(sg):
                M = SG_M[sg]
                r0 = sg * 128
                dots_t = dots_tiles[sg]
                for ci, (goff, w) in enumerate(GCHUNKS):
                    gt = gt_tiles.pop((sg, ci))
                    for j in range(w):
                        g = goff + j
                        dps = psDp.tile([128, 512], F32, tag="dps")
                        for dblk in range(10):
                            nc.tensor.matmul(
                                dps[:M, 0:128],
                                lhsT=predT[:, dblk, r0 : r0 + M],
                                rhs=gt[:, dblk, j * 128 : (j + 1) * 128],
                                start=(dblk == 0),
                                stop=(dblk == 9),
                            )
                        # diag extract: accum((psum * 1.0) * I) -> dots col
                        nc.vector.scalar_tensor_tensor(
                            scr128[:M, :],
                            dps[:M, 0:128],
                            1.0,
                            ident_sb[:M, :],
                            op0=Alu.mult,
                            op1=Alu.mult,
                            accum_out=dots_t[:M, g : g + 1],
                        )
                emit_gathers(len(GCHUNKS) * (sg + 1) + 3)

            # ---- phase 1: predT = Wk[s] @ ctx^T + b (single-pass fp16) ----
            for s in range(S):
                n = NS[s]
                o = SOFF[s]
                wk_t = wkp.tile([128, 10, 10, 128], FP16, tag="wk")
                for dch in range(2):
                    nc.sync.dma_start(
                        wk_t[:, 5 * dch : 5 * dch + 5, :, :],
                        wk5[:, s, 5 * dch : 5 * dch + 5, :, :],
                    )
                for eblk in range(10):
                    ps = psAp.tile([128, 512], F32, tag="ps")
                    for dblk in range(10):
                        nc.tensor.matmul(
                            ps[:, 0:n],
                            lhsT=wk_t[:, eblk, dblk, :],
                            rhs=ctx_sb[:, dblk, o : o + n],
                            start=(dblk == 0),
                            stop=False,
                        )
                    # bias: predT[e, r] += b[e] * 1
                    nc.tensor.matmul(
                        ps[:, 0:n],
                        lhsT=wkb_sb[0:1, s, eblk, :],
                        rhs=ones16[0:1, 0:n],
                        start=False,
                        stop=True,
                    )
                    nc.scalar.copy(predT[:, eblk, o : o + n], ps[:, 0:n])
                for sg in SG_AFTER_S[s]:
                    emit_phase2(sg)

            # ---- CE finale (deferred): one Exp table load, one Ln ----
            for sg in range(NSG):
                dots_t = dots_tiles[sg]
                negm = smallp.tile([128, 1], F32, tag="negm")
                nc.vector.tensor_reduce(
                    negm[:, :], dots_t[:, :], Ax.X, Alu.max, negate=True
                )
                e_t = smallp.tile([128, NDOT], F32, tag="et")
                nc.scalar.activation(
                    e_t[:, :],
                    dots_t[:, :],
                    Act.Exp,
                    bias=negm[:, 0:1],
                    scale=1.0,
                    accum_out=ss9[:, sg : sg + 1],
                )
                nc.vector.tensor_tensor(
                    pm9[:, sg : sg + 1], dots_t[:, 0:1], negm[:, :], Alu.add
                )
                maxneg = smallp.tile([128, 1], F32, tag="maxneg")
                nc.vector.tensor_reduce(
                    maxneg[:, :], dots_t[:, 1:NDOT], Ax.X, Alu.max
                )
                nc.vector.tensor_tensor(
                    lc18[:, NSG + sg : NSG + sg + 1],
                    dots_t[:, 0:1],
                    maxneg[:, :],
                    Alu.is_ge,
                )
            # loss_r = ln(ss) - (pos - m)
            ln9 = smallp.tile([128, NSG], F32, tag="ln9")
            nc.scalar.activation(ln9[:, :], ss9[:, :], Act.Ln)
            nc.vector.tensor_tensor(
                lc18[:, 0:NSG], ln9[:, :], pm9[:, :], Alu.subtract
            )
            # zero the 32 invalid rows of supergroup 8
            nc.vector.memset(lc18[96:128, 8:9], 0.0)
            nc.vector.memset(lc18[96:128, NSG + 8 : NSG + 9], 0.0)

            # ---- final partition reduce: [128,18] -> [1,18] -> [1,2] ----
            psf = psFp.tile([1, 2 * NSG], F32, tag="psf")
            nc.tensor.matmul(
                psf[:, :], lhsT=ones_sb[:, 0:1], rhs=lc18[:, :], start=True,
                stop=True,
            )
            sum18 = smallp.tile([1, 2 * NSG], F32, tag="sum18")
            nc.vector.tensor_copy(sum18[:, :], psf[:, :])
            outsb = smallp.tile([1, 2], F32, tag="outsb")
            nc.vector.tensor_reduce(
                outsb[:, 0:2],
                sum18[:, :].rearrange("p (a b) -> p a b", a=2),
                Ax.X,
                Alu.add,
            )
            nc.sync.dma_start(out[:, :], outsb[:, :])

    nc.compile()
    return nc


def _row_targets(core: int, neg_idx: np.ndarray) -> np.ndarray:
    """[NR, 17] int array: flat enc index of positive + 16 negatives per row."""
    tg = np.zeros((NR, NDOT), np.int64)
    ri = 0
    for s in range(S):
        rows = 6 - s
        for b in range(BSH):
            bg = core * BSH + b
            for r in range(rows):
                for c7 in range(G):
                    tg[ri, 0] = bg * G * G + (s + 1 + r) * G + c7
                    tg[ri, 1:] = neg_idx[bg, s, r, c7]
                    ri += 1
    assert ri == NR
    return tg


def _build_idx(core: int, neg_idx: np.ndarray) -> np.ndarray:
    """int16 [128, IDX_TOT//16] gather-index tensor in SWDGE wrap layout."""
    tg = _row_targets(core, neg_idx)
    tg_pad = np.zeros((NSG * 128, NDOT), np.int64)
    tg_pad[:NR] = tg
    # list position sg*2176 + g*128 + p  ->  target of (row sg*128+p, dot g)
    lst = tg_pad.reshape(NSG, 128, NDOT).transpose(0, 2, 1).reshape(-1)
    arr = lst.astype(np.int16).reshape(-1, 16).T  # [16, IDX_TOT//16]
    return np.ascontiguousarray(np.tile(arr, (8, 1)))  # [128, ...]


def _prep_in_maps(contexts, encodings, Wk_w, Wk_b, neg_idx):
    contexts = np.ascontiguousarray(np.asarray(contexts, np.float32))
    encodings = np.ascontiguousarray(np.asarray(encodings, np.float32))
    Wk_w = np.ascontiguousarray(np.asarray(Wk_w, np.float32))
    Wk_b = np.ascontiguousarray(np.asarray(Wk_b, np.float32))
    neg_idx = np.asarray(neg_idx)

    ench = np.ascontiguousarray(
        encodings.reshape(B * G * G, D).astype(np.float16)
    )
    # wk5[di, s, eo, do, ei] = Wk_w[s, eo*128+ei, do*128+di]
    wk5 = np.ascontiguousarray(
        Wk_w.reshape(S, 10, 128, 10, 128)
        .transpose(4, 0, 1, 3, 2)
        .astype(np.float16)
    )
    wkb = np.ascontiguousarray(
        Wk_b.reshape(1, S, 10, 128).astype(np.float16)
    )
    identm = np.ascontiguousarray(np.eye(128, dtype=np.float16))

    in_maps = []
    for c in range(NCORES):
        bs = slice(c * BSH, (c + 1) * BSH)
        ctx_rows = np.concatenate(
            [contexts[bs, : 6 - s].reshape(-1, D) for s in range(S)], axis=0
        )
        in_maps.append(
            {
                "ctxT": np.ascontiguousarray(ctx_rows.T.astype(np.float16)),
                "wk5": wk5,
                "wkb": wkb,
                "ench": ench,
                "ident": identm,
                "idx": _build_idx(c, neg_idx),
            }
        )
    return in_maps


def kernel(contexts, encodings, Wk_w, Wk_b, neg_idx, _trace=False):
    in_maps = _prep_in_maps(contexts, encodings, Wk_w, Wk_b, neg_idx)
    nc = build_nc()
    res = run_bass_kernel_spmd(nc, in_maps, list(range(NCORES)), trace=_trace)
    LAST_RUN["exec_time_ns"] = res.exec_time_ns
    LAST_RUN["results"] = res.results
    loss = np.float32(0.0)
    corr = np.float32(0.0)
    for o in res.results:
        loss += np.float32(o["out"][0, 0])
        corr += np.float32(o["out"][0, 1])
    return (
        np.float32(loss / np.float32(N_PREDS)),
        np.float32(corr / np.float32(N_PREDS)),
    )


# revision 17
# speedup vs baseline: 1.6932x; 1.1272x over previous
"""Trainium2 Bass kernel for the CPC loss problem (nn_CPC_85117661872355).

Strategy (data-parallel over batch B across 8 cores):
  - Each core handles 8 of the 64 batch elements: 1120 prediction rows.
  - Phase 1 computes pred TRANSPOSED: predT[e, r] = sum_d Wk[s][e,d] ctx[r,d]
    + b[e], as a single-pass fp16 matmul (Wk blocks stationary, ctx^T rows
    moving, fp32 PSUM accumulate).  E lands on partitions, so the ragged
    per-s row groups go on the free axis and no repack is needed.
  - All 17 logits per row (1 positive + 16 negatives) are dot products
    pred_row . enc_flat[idx], contracted over E.  Target vectors are fetched
    with SWDGE dma_gather(transpose=True) from an fp16 copy of the encoding
    table, which lands them E-on-partitions: gtT[e_sub, e_blk, j].  Each
    128-dot group is then a 10-matmul PE accumulation
    out[r, j] = sum_e predT[e, r] gtT[e, j] whose DIAGONAL holds the dots;
    a fused DVE scalar_tensor_tensor against a host-supplied identity
    extracts diag+accumulates in one [128,128] op.  This moves the 25M
    multiply-adds of the dot products from DVE (1 elem/cycle) to the PE.
  - Gathering the positive through the same path keeps bitwise ties when a
    negative index collides with the positive (jnp.argmax first-index
    tie-break).
  - Dots tiles for all 9 supergroups are kept; softmax-CE runs once at the
    end (one Exp table load, one Ln), accumulating loss/correct per
    partition; a ones-matmul reduces to [1,2] per core; host sums cores.
"""

import functools

import numpy as np

import concourse.bass as bass
import concourse.mybir as mybir
import concourse.tile as tile
from concourse import bacc
from concourse.bass_utils import run_bass_kernel_spmd

F32 = mybir.dt.float32
FP16 = mybir.dt.float16

B, G, D = 64, 7, 1280
S, NEG = 5, 16
NCORES = 8
BSH = B // NCORES  # 8
NS = [BSH * (6 - s) * G for s in range(S)]  # [336, 280, 224, 168, 112]
SOFF = [0]
for n in NS:
    SOFF.append(SOFF[-1] + n)
NR = SOFF[-1]  # 1120 rows per core
NSG = 9  # supergroups of 128 rows
NDOT = 17  # 1 positive + 16 negatives
GCHUNKS = [(0, 6), (6, 6), (12, 5)]  # gather (goff, width) per supergroup
IDX_PER_SG = NDOT * 128  # 2176
IDX_TOT = NSG * IDX_PER_SG  # 19584
N_PREDS = B * G * 20  # 8960

# Results of the last device run (for test harness introspection)
LAST_RUN = {}


@functools.lru_cache(maxsize=1)
def build_nc() -> bass.Bass:
    nc = bacc.Bacc(
        "TRN2",
        target_bir_lowering=False,
        debug=False,
        num_devices=NCORES,
        num_swdge_queues=2,
    )
    # ctxT: [d, r] with d split [128 d_sub, 10 d_blk]
    ctxT = nc.declare_dram_parameter("ctxT", [D, NR], FP16, isOutput=False)
    # wk5: [128 d_in_sub, S, 10 d_out_blk(e), 10 d_in_blk, 128 e_sub]
    # element [di, s, eo, do, ei] = Wk_w[s, eo*128+ei, do*128+di]
    wk5 = nc.declare_dram_parameter("wk5", [128, S, 10, 10, 128], FP16,
                                    isOutput=False)
    wkb = nc.declare_dram_parameter("wkb", [1, S, 10, 128], FP16,
                                    isOutput=False)
    ench = nc.declare_dram_parameter("ench", [B * G * G, D], FP16,
                                     isOutput=False)
    ident = nc.declare_dram_parameter("ident", [128, 128], FP16,
                                      isOutput=False)
    idx = nc.declare_dram_parameter(
        "idx", [128, IDX_TOT // 16], mybir.dt.int16, isOutput=False
    )
    out = nc.declare_dram_parameter("out", [1, 2], F32, isOutput=True)

    Alu = mybir.AluOpType
    Act = mybir.ActivationFunctionType
    Ax = mybir.AxisListType

    # supergroups fully covered after each s finishes phase 1
    SG_AFTER_S = [[0, 1], [2, 3], [4, 5], [6], [7, 8]]
    SG_M = [128] * 8 + [96]  # valid rows per supergroup

    with tile.TileContext(nc) as tc:
        with (
            tc.tile_pool(name="const", bufs=1) as constp,
            tc.tile_pool(name="wk", bufs=2) as wkp,
            tc.tile_pool(name="gath", bufs=3) as gathp,
            tc.tile_pool(name="gath1", bufs=2) as gath1p,
            tc.tile_pool(name="dots", bufs=NSG) as dotsp,
            tc.tile_pool(name="small", bufs=4) as smallp,
            tc.tile_pool(name="acc", bufs=1) as accp,
            tc.tile_pool(name="psA", bufs=2, space="PSUM") as psAp,
            tc.tile_pool(name="psD", bufs=3, space="PSUM") as psDp,
            tc.tile_pool(name="psF", bufs=1, space="PSUM") as psFp,
        ):
            # ---- constants ----
            idx_sb = constp.tile([128, IDX_TOT // 16], mybir.dt.int16,
                                 tag="idx")
            nc.sync.dma_start(idx_sb[:, :], idx[:, :])
            ident_sb = constp.tile([128, 128], FP16, tag="ident")
            nc.sync.dma_start(ident_sb[:, :], ident[:, :])
            ones_sb = constp.tile([128, 1], F32, tag="ones")
            nc.vector.memset(ones_sb[:, :], 1.0)
            ones16 = constp.tile([1, 512], FP16, tag="ones16")
            nc.vector.memset(ones16[:, :], 1.0)
            wkb_sb = constp.tile([1, S, 10, 128], FP16, tag="wkb")
            nc.sync.dma_start(wkb_sb[:, :, :, :], wkb[:, :, :, :])

            # resident fp16 ctx^T: [128 d_sub, 10 d_blk, NR rows]
            ctx_sb = constp.tile([128, 10, NR], FP16, tag="ctx")
            ctx_r = ctxT[:, :].rearrange("(do di) r -> di do r", di=128)
            nc.sync.dma_start(ctx_sb[:, :, 0:560], ctx_r[:, :, 0:560])
            nc.sync.dma_start(ctx_sb[:, :, 560:NR], ctx_r[:, :, 560:NR])

            # predT resident: [128 e_sub, 10 e_blk, NR rows] fp16
            predT = constp.tile([128, 10, NR], FP16, tag="predT")

            # CE stat accumulators
            ss9 = accp.tile([128, NSG], F32, tag="ss9")
            pm9 = accp.tile([128, NSG], F32, tag="pm9")
            lc18 = accp.tile([128, 2 * NSG], F32, tag="lc18")
            scr128 = accp.tile([128, 128], FP16, tag="scr128")

            # ---- gathers (transposed): issued with small lookahead ----
            ench_ap = ench[:, :]
            gt_tiles = {}
            gcursor = [0]

            def emit_gathers(upto):
                while gcursor[0] < min(upto, NSG * len(GCHUNKS)):
                    k = gcursor[0]
                    sg, ci = divmod(k, len(GCHUNKS))
                    goff, w = GCHUNKS[ci]
                    nidx = w * 128
                    pool = gathp if w == 6 else gath1p
                    gt = pool.tile([128, 10, nidx], FP16, tag=f"gt{w}")
                    pos0 = sg * IDX_PER_SG + goff * 128
                    nc.gpsimd.dma_gather(
                        gt[:, :, :],
                        ench_ap,
                        idx_sb[:, pos0 // 16 : (pos0 + nidx) // 16],
                        nidx,
                        nidx,
                        D,
                        transpose=True,
                        queue_num=(k % 8) // 4,
                    )
                    gt_tiles[(sg, ci)] = gt
                    gcursor[0] += 1

            emit_gathers(3)

            dots_tiles = [
                dotsp.tile([128, NDOT], F32, tag="dots", name=f"dots{i}")
                for i in range(NSG)
            ]
            # rows 96.. of supergroup 8 never get extracts; keep them finite
            nc.vector.memset(dots_tiles[8][96:128, :], 0.0)

            def emit_phase2(sg):
                M = SG_M[sg]
                r0 = sg * 128
                dots_t = dots_tiles[sg]
                for ci, (goff, w) in enumerate(GCHUNKS):
                    gt = gt_tiles.pop((sg, ci))
                    for j in range(w):
                        g = goff + j
                        dps = psDp.tile([128, 512], F32, tag="dps")
                        for dblk in range(10):
                            nc.tensor.matmul(
                                dps[:M, 0:128],
                                lhsT=predT[:, dblk, r0 : r0 + M],
                                rhs=gt[:, dblk, j * 128 : (j + 1) * 128],
                                start=(dblk == 0),
                                stop=(dblk == 9),
                            )
                        # diag extract: accum((psum * 1.0) * I) -> dots col
                        nc.vector.scalar_tensor_tensor(
                            scr128[:M, :],
                            dps[:M, 0:128],
                            1.0,
                            ident_sb[:M, :],
                            op0=Alu.mult,
                            op1=Alu.mult,
                            accum_out=dots_t[:M, g : g + 1],
                        )
                # per-sg CE stats inline (keeps the tail short)
                negm = smallp.tile([128, 1], F32, tag="negm")
                nc.vector.tensor_reduce(
                    negm[:, :], dots_t[:, :], Ax.X, Alu.max, negate=True
                )
                e_t = smallp.tile([128, NDOT], F32, tag="et")
                nc.scalar.activation(
                    e_t[:, :],
                    dots_t[:, :],
                    Act.Exp,
                    bias=negm[:, 0:1],
                    scale=1.0,
                    accum_out=ss9[:, sg : sg + 1],
                )
                nc.vector.tensor_tensor(
                    pm9[:, sg : sg + 1], dots_t[:, 0:1], negm[:, :], Alu.add
                )
                maxneg = smallp.tile([128, 1], F32, tag="maxneg")
                nc.vector.tensor_reduce(
                    maxneg[:, :], dots_t[:, 1:NDOT], Ax.X, Alu.max
                )
                nc.vector.tensor_tensor(
                    lc18[:, NSG + sg : NSG + sg + 1],
                    dots_t[:, 0:1],
                    maxneg[:, :],
                    Alu.is_ge,
                )
                emit_gathers(len(GCHUNKS) * (sg + 1) + 3)

            # ---- phase 1: predT = Wk[s] @ ctx^T + b (single-pass fp16) ----
            for s in range(S):
                n = NS[s]
                o = SOFF[s]
                wk_t = wkp.tile([128, 10, 10, 128], FP16, tag="wk")
                for dch in range(2):
                    nc.sync.dma_start(
                        wk_t[:, 5 * dch : 5 * dch + 5, :, :],
                        wk5[:, s, 5 * dch : 5 * dch + 5, :, :],
                    )
                for eblk in range(10):
                    ps = psAp.tile([128, 512], F32, tag="ps")
                    for dblk in range(10):
                        nc.tensor.matmul(
                            ps[:, 0:n],
                            lhsT=wk_t[:, eblk, dblk, :],
                            rhs=ctx_sb[:, dblk, o : o + n],
                            start=(dblk == 0),
                            stop=False,
                        )
                    # bias: predT[e, r] += b[e] * 1
                    nc.tensor.matmul(
                        ps[:, 0:n],
                        lhsT=wkb_sb[0:1, s, eblk, :],
                        rhs=ones16[0:1, 0:n],
                        start=False,
                        stop=True,
                    )
                    nc.scalar.copy(predT[:, eblk, o : o + n], ps[:, 0:n])
                for sg in SG_AFTER_S[s]:
                    emit_phase2(sg)

            # ---- CE finale: loss_r = ln(ss) - (pos - m) ----
            ln9 = smallp.tile([128, NSG], F32, tag="ln9")
            nc.scalar.activation(ln9[:, :], ss9[:, :], Act.Ln)
            nc.vector.tensor_tensor(
                lc18[:, 0:NSG], ln9[:, :], pm9[:, :], Alu.subtract
            )
            # zero the 32 invalid rows of supergroup 8
            nc.vector.memset(lc18[96:128, 8:9], 0.0)
            nc.vector.memset(lc18[96:128, NSG + 8 : NSG + 9], 0.0)

            # ---- final partition reduce: [128,18] -> [1,18] -> [1,2] ----
            psf = psFp.tile([1, 2 * NSG], F32, tag="psf")
            nc.tensor.matmul(
                psf[:, :], lhsT=ones_sb[:, 0:1], rhs=lc18[:, :], start=True,
                stop=True,
            )
            sum18 = smallp.tile([1, 2 * NSG], F32, tag="sum18")
            nc.vector.tensor_copy(sum18[:, :], psf[:, :])
            outsb = smallp.tile([1, 2], F32, tag="outsb")
            nc.vector.tensor_reduce(
                outsb[:, 0:2],
                sum18[:, :].rearrange("p (a b) -> p a b", a=2),
                Ax.X,
                Alu.add,
            )
            nc.sync.dma_start(out[:, :], outsb[:, :])

    nc.compile()
    return nc


def _row_targets(core: int, neg_idx: np.ndarray) -> np.ndarray:
    """[NR, 17] int array: flat enc index of positive + 16 negatives per row."""
    tg = np.zeros((NR, NDOT), np.int64)
    ri = 0
    for s in range(S):
        rows = 6 - s
        for b in range(BSH):
            bg = core * BSH + b
            for r in range(rows):
                for c7 in range(G):
                    tg[ri, 0] = bg * G * G + (s + 1 + r) * G + c7
                    tg[ri, 1:] = neg_idx[bg, s, r, c7]
                    ri += 1
    assert ri == NR
    return tg


def _build_idx(core: int, neg_idx: np.ndarray) -> np.ndarray:
    """int16 [128, IDX_TOT//16] gather-index tensor in SWDGE wrap layout."""
    tg = _row_targets(core, neg_idx)
    tg_pad = np.zeros((NSG * 128, NDOT), np.int64)
    tg_pad[:NR] = tg
    # list position sg*2176 + g*128 + p  ->  target of (row sg*128+p, dot g)
    lst = tg_pad.reshape(NSG, 128, NDOT).transpose(0, 2, 1).reshape(-1)
    arr = lst.astype(np.int16).reshape(-1, 16).T  # [16, IDX_TOT//16]
    return np.ascontiguousarray(np.tile(arr, (8, 1)))  # [128, ...]


def _prep_in_maps(contexts, encodings, Wk_w, Wk_b, neg_idx):
    contexts = np.ascontiguousarray(np.asarray(contexts, np.float32))
    encodings = np.ascontiguousarray(np.asarray(encodings, np.float32))
    Wk_w = np.ascontiguousarray(np.asarray(Wk_w, np.float32))
    Wk_b = np.ascontiguousarray(np.asarray(Wk_b, np.float32))
    neg_idx = np.asarray(neg_idx)

    ench = np.ascontiguousarray(
        encodings.reshape(B * G * G, D).astype(np.float16)
    )
    # wk5[di, s, eo, do, ei] = Wk_w[s, eo*128+ei, do*128+di]
    wk5 = np.ascontiguousarray(
        Wk_w.reshape(S, 10, 128, 10, 128)
        .transpose(4, 0, 1, 3, 2)
        .astype(np.float16)
    )
    wkb = np.ascontiguousarray(
        Wk_b.reshape(1, S, 10, 128).astype(np.float16)
    )
    identm = np.ascontiguousarray(np.eye(128, dtype=np.float16))

    in_maps = []
    for c in range(NCORES):
        bs = slice(c * BSH, (c + 1) * BSH)
        ctx_rows = np.concatenate(
            [contexts[bs, : 6 - s].reshape(-1, D) for s in range(S)], axis=0
        )
        in_maps.append(
            {
                "ctxT": np.ascontiguousarray(ctx_rows.T.astype(np.float16)),
                "wk5": wk5,
                "wkb": wkb,
                "ench": ench,
                "ident": identm,
                "idx": _build_idx(c, neg_idx),
            }
        )
    return in_maps


def kernel(contexts, encodings, Wk_w, Wk_b, neg_idx, _trace=False):
    in_maps = _prep_in_maps(contexts, encodings, Wk_w, Wk_b, neg_idx)
    nc = build_nc()
    res = run_bass_kernel_spmd(nc, in_maps, list(range(NCORES)), trace=_trace)
    LAST_RUN["exec_time_ns"] = res.exec_time_ns
    LAST_RUN["results"] = res.results
    loss = np.float32(0.0)
    corr = np.float32(0.0)
    for o in res.results:
        loss += np.float32(o["out"][0, 0])
        corr += np.float32(o["out"][0, 1])
    return (
        np.float32(loss / np.float32(N_PREDS)),
        np.float32(corr / np.float32(N_PREDS)),
    )
